# revision 25
# baseline (speedup 1.0000x reference)
"""Multi-head attention (B=4, S=1024, D=1024, H=16) on 8 TRN2 NeuronCores.

Sharding: core c = (batch b = c//2, head-group g = c%2). Each core computes
one batch and 8 of the 16 heads (W columns g*512:(g+1)*512), so per-core
inputs are query[b]/key[b]/value[b] (1024x1024) and W[:, cols] (1024x512).

Per-core kernel:
  - x^T via PE transposes (fp32 has no DMA-transpose path)
  - Q^T/K^T = W^T @ x^T and V = x^T.T @ W as float32r matmuls (full PE
    rate; operands must be produced by an f32r-rounding instruction per
    the BIR verifier), pipelined per 128-dcol slice with the head pairs
    that consume them
  - scores [q,k] once per q-tile; exp + row-sum fused on ACT (accum_out,
    bf16 out); no max-subtraction needed (scores ~ N(0,1), exp safe)
  - attn out = exp * (1/rowsum) on GpSimd, DMA'd in 2-row-block groups
    issued alternately from SP and GpSimd (SWDGE) to parallelize
    descriptor generation
  - exp^T for the context contraction via PE block transposes (emitted one
    q-tile behind the score matmuls so the in-order PE stream never stalls
    on ACT), drained by DVE
  - context as ctx^T = V_h^T(lhsT) @ exp^T(rhs) in bf16, transposed back
    via PE, scaled by 1/rowsum at drain
  - V bias is folded on the host (attn rows sum to 1 => ctx += bv exactly)
"""

import numpy as np
from contextlib import ExitStack

import concourse.bass as bass
import concourse.tile as tile
from concourse import bacc, mybir
from concourse.bass_utils import run_bass_kernel_spmd
from concourse.masks import make_identity

P = 128
S = 1024      # tokens per batch
D = 1024      # model dim
HC = 8        # heads per core
DH = 64       # head dim
WC = 512      # W columns per core
KS = D // P   # 8
MS = WC // P  # 4
TS = S // P   # 8
NT = 512      # matmul moving-dim tile

F32 = mybir.dt.float32
F32R = mybir.dt.float32r
BF16 = mybir.dt.bfloat16
EXP = mybir.ActivationFunctionType.Exp


def _body(ctx, tc, nc, xq_d, xk_d, xv_d, wq_d, wk_d, wv_d, bq_d, bk_d,
          ctx_d, attn_d):
    consts = ctx.enter_context(tc.tile_pool(name="consts", bufs=1))
    wpool = ctx.enter_context(tc.tile_pool(name="w", bufs=2))
    wstage = ctx.enter_context(tc.tile_pool(name="wstage", bufs=2))
    xrow = ctx.enter_context(tc.tile_pool(name="xrow", bufs=2))
    xt = ctx.enter_context(tc.tile_pool(name="xt", bufs=2))
    proj = ctx.enter_context(tc.tile_pool(name="proj", bufs=1))
    hp = ctx.enter_context(tc.tile_pool(name="hp", bufs=2))
    psA = ctx.enter_context(tc.tile_pool(name="psA", bufs=3, space="PSUM"))
    psT = ctx.enter_context(tc.tile_pool(name="psT", bufs=2, space="PSUM"))

    ident = consts.tile([P, P], F32)
    make_identity(nc, ident)
    identb = consts.tile([P, P], BF16)
    nc.vector.tensor_copy(out=identb[:], in_=ident[:])

    bqt = consts.tile([P, MS], F32)
    nc.sync.dma_start(bqt[:], bq_d.rearrange("(o p) -> p o", p=P))
    bkt = consts.tile([P, MS], F32)
    nc.sync.dma_start(bkt[:], bk_d.rearrange("(o p) -> p o", p=P))

    def load_w(w_d):
        # fp32r matmul operands must be explicitly rounded to fp32r by their
        # producer (BIR verifier rule): stage the f32 DMA, round via GpSimd.
        w = wpool.tile([P, KS, WC], F32R, tag="w")
        wre = w_d.rearrange("(ko p) m -> p ko m", p=P)
        for g4 in range(4):
            st = wstage.tile([P, KS // 4, WC], F32, tag="wstage")
            nc.sync.dma_start(st[:], wre[:, g4 * 2:(g4 + 1) * 2, :])
            nc.gpsimd.tensor_copy(out=w[:, g4 * 2:(g4 + 1) * 2, :], in_=st[:])
        return w

    def load_xt(x_d):
        """x [S, D] -> x^T in SBUF as [p, ds, s] with d = ds*128 + p."""
        t = xt.tile([P, KS, S], F32R, tag="xt")
        xre = x_d.rearrange("(t p) d -> p t d", p=P)
        for tg in range(TS // 2):
            row = xrow.tile([P, 2, D], F32, tag="xrow")
            nc.sync.dma_start(row[:], xre[:, tg * 2:(tg + 1) * 2, :])
            for ti in range(2):
                t_ = tg * 2 + ti
                for g4 in range(2):
                    pt = psT.tile([P, 4 * P], F32, tag="pst")
                    for j in range(4):
                        ds = g4 * 4 + j
                        nc.tensor.transpose(
                            pt[:, j * P:(j + 1) * P],
                            row[:, ti, ds * P:(ds + 1) * P], ident[:])
                    nc.vector.tensor_copy(
                        out=t[:, g4 * 4:(g4 + 1) * 4, t_ * P:(t_ + 1) * P],
                        in_=pt[:].rearrange("p (a b) -> p a b", a=4))
        return t

    # --- V = (xv @ Wv) in natural [token, dcol] layout, bf16 ---
    wv = load_w(wv_d)
    xtv = load_xt(xv_d)
    vb = proj.tile([P, TS, WC], BF16, tag="vb")
    for t_ in range(TS):
        ps = psA.tile([P, 2 * NT], F32, tag="mm")
        for ko in range(KS):
            nc.tensor.matmul(ps[:, :NT], xtv[:, ko, t_ * P:(t_ + 1) * P],
                             wv[:, ko, :],
                             start=(ko == 0), stop=(ko == KS - 1))
        nc.vector.tensor_copy(out=vb[:, t_, :], in_=ps[:, :NT])

    wq = load_w(wq_d)
    xtq = load_xt(xq_d)
    wk = load_w(wk_d)
    xtk = load_xt(xk_d)

    def project_m(w, xtx, bias, m, name):
        """One [dcol=128] slice of Q^T/K^T: [128, 1024] f32r, bias folded."""
        out = proj.tile([P, S], F32R, tag=name, name=f"{name}{m}", bufs=2)
        ps = psA.tile([P, 2 * NT], F32, tag="mm")
        for nt_ in range(S // NT):
            for ko in range(KS):
                nc.tensor.matmul(
                    ps[:, nt_ * NT:(nt_ + 1) * NT],
                    w[:, ko, m * P:(m + 1) * P],
                    xtx[:, ko, nt_ * NT:(nt_ + 1) * NT],
                    start=(ko == 0), stop=(ko == KS - 1))
        nc.vector.tensor_scalar_add(out[:], ps[:], bias[:, m:m + 1])
        return out

    ctxf = proj.tile([P, TS, WC], F32, tag="ctxf")

    def head(h, qTm, kTm):
        po = (h % 2) * DH
        qh = qTm[po:po + DH, :]   # [64, 1024]
        kh = kTm[po:po + DH, :]

        recip = hp.tile([P, TS], F32, tag="recip")
        exT = hp.tile([P, TS, S], BF16, tag="exT", bufs=1)
        atre = attn_d[h].rearrange("(t p) k -> p t k", p=P)

        # scores [q, k] -> exp(bf16) + rowsum -> (a) normalize -> attn out,
        # (b) PE-transpose into exT [k, q] for the context matmul.
        # Transposes for qt are emitted AFTER qt+1's matmuls so the in-order
        # PE stream never waits on ACT's exp.
        def transposes(qt, exb):
            for g4 in range(2):
                pt = psT.tile([P, 4 * P], BF16, tag="pst")
                for j in range(4):
                    kt = g4 * 4 + j
                    nc.tensor.transpose(pt[:, j * P:(j + 1) * P],
                                        exb[:, kt * P:(kt + 1) * P],
                                        identb[:])
                nc.vector.tensor_copy(
                    out=exT[:, g4 * 4:(g4 + 1) * 4, qt * P:(qt + 1) * P],
                    in_=pt[:].rearrange("p (a b) -> p a b", a=4))

        pending = None
        for qg in range(TS // 2):
            dma_eng = nc.sync if qg % 2 == 0 else nc.gpsimd
            at2 = hp.tile([P, 2, S], F32, tag="at")
            Lp = hp.tile([P, 2], F32, tag="Lp")
            exbs = []
            for q2 in range(2):
                qt = qg * 2 + q2
                ps = psA.tile([P, 2 * NT], F32, tag="mm")
                for nk in range(2):
                    nc.tensor.matmul(ps[:, nk * NT:(nk + 1) * NT],
                                     qh[:, qt * P:(qt + 1) * P],
                                     kh[:, nk * NT:(nk + 1) * NT],
                                     start=True, stop=True)
                exb = hp.tile([P, S], BF16, tag="ex", bufs=4)
                nc.scalar.activation(out=exb[:], in_=ps[:], func=EXP,
                                     scale=0.125, accum_out=Lp[:, q2:q2 + 1])
                exbs.append((qt, exb))
                if pending is not None:
                    transposes(*pending)
                pending = (qt, exb)
            nc.vector.reciprocal(out=recip[:, qg * 2:(qg + 1) * 2], in_=Lp[:])
            for q2, (qt, exb) in enumerate(exbs):
                nc.gpsimd.tensor_scalar_mul(at2[:, q2, :], exb[:],
                                            recip[:, qt:qt + 1])
            dma_eng.dma_start(atre[:, qg * 2:(qg + 1) * 2, :], at2[:])
        if pending is not None:
            transposes(*pending)

        # ctx^T = V_h^T @ expT (bf16), then PE-transpose back + 1/L scale
        for nq in range(2):
            psc_full = psA.tile([P, 2 * NT], F32, tag="mm", name="psc")
            psc = psc_full[:, :NT]
            for kt in range(TS):
                nc.tensor.matmul(psc[:DH], vb[:, kt, h * DH:(h + 1) * DH],
                                 exT[:, kt, nq * NT:(nq + 1) * NT],
                                 start=(kt == 0), stop=(kt == TS - 1))
            ctb = hp.tile([DH, NT], BF16, tag="ctb")
            nc.vector.tensor_copy(out=ctb[:], in_=psc[:DH])
            for j in range(4):
                pt2 = psT.tile([P, P], BF16, tag="pst")
                nc.tensor.transpose(pt2[:, :DH], ctb[:, j * P:(j + 1) * P],
                                    identb[:DH, :DH])
                qt = nq * 4 + j
                nc.vector.tensor_scalar_mul(
                    ctxf[:, qt, h * DH:(h + 1) * DH], pt2[:, :DH],
                    recip[:, qt:qt + 1])

    # m-pipelined: project one dcol-slice of Q^T/K^T, then run its 2 heads,
    # so the scalar engine starts exp work while later slices still project.
    for m in range(MS):
        qTm = project_m(wq, xtq, bqt, m, "qTm")
        kTm = project_m(wk, xtk, bkt, m, "kTm")
        head(2 * m, qTm, kTm)
        head(2 * m + 1, qTm, kTm)

    nc.sync.dma_start(ctx_d.rearrange("(t p) d -> p t d", p=P), ctxf[:])


_NC_CACHE = None


def build_program():
    global _NC_CACHE
    if _NC_CACHE is not None:
        return _NC_CACHE
    nc = bacc.Bacc("TRN2", target_bir_lowering=False, debug=False)
    xq = nc.dram_tensor("xq", [S, D], F32, kind="ExternalInput").ap()
    xk = nc.dram_tensor("xk", [S, D], F32, kind="ExternalInput").ap()
    xv = nc.dram_tensor("xv", [S, D], F32, kind="ExternalInput").ap()
    wq = nc.dram_tensor("wq", [D, WC], F32, kind="ExternalInput").ap()
    wk = nc.dram_tensor("wk", [D, WC], F32, kind="ExternalInput").ap()
    wv = nc.dram_tensor("wv", [D, WC], F32, kind="ExternalInput").ap()
    bq = nc.dram_tensor("bq", [WC], F32, kind="ExternalInput").ap()
    bk = nc.dram_tensor("bk", [WC], F32, kind="ExternalInput").ap()
    ctx_d = nc.dram_tensor("ctx", [S, WC], F32, kind="ExternalOutput").ap()
    attn_d = nc.dram_tensor("attn", [HC, S, S], F32,
                            kind="ExternalOutput").ap()
    with tile.TileContext(nc) as tc, ExitStack() as ctx:
        _body(ctx, tc, nc, xq, xk, xv, wq, wk, wv, bq, bk, ctx_d, attn_d)
    nc.compile()
    _NC_CACHE = nc
    return nc


def make_in_maps(query, key, value, Wq, Wk, Wv, bq, bk):
    in_maps = []
    for c in range(8):
        b, g = divmod(c, 2)
        cs = slice(g * WC, (g + 1) * WC)
        in_maps.append({
            "xq": np.ascontiguousarray(query[b]),
            "xk": np.ascontiguousarray(key[b]),
            "xv": np.ascontiguousarray(value[b]),
            "wq": np.ascontiguousarray(Wq[:, cs]),
            "wk": np.ascontiguousarray(Wk[:, cs]),
            "wv": np.ascontiguousarray(Wv[:, cs]),
            "bq": np.ascontiguousarray(bq[cs]),
            "bk": np.ascontiguousarray(bk[cs]),
        })
    return in_maps


def gather(results, bv):
    B, H = 4, 16
    context = np.empty((B, S, D), np.float32)
    attn = np.empty((H * B, S, S), np.float32)
    for c in range(8):
        b, g = divmod(c, 2)
        context[b, :, g * WC:(g + 1) * WC] = results[c]["ctx"]
        for hl in range(HC):
            attn[(g * HC + hl) * B + b] = results[c]["attn"][hl]
    context += np.asarray(bv, np.float32)  # exact: attn rows sum to 1
    return context, attn


def kernel(query, key, value, Wq, bq, Wk, bk, Wv, bv, _trace=False):
    query = np.asarray(query, np.float32)
    key = np.asarray(key, np.float32)
    value = np.asarray(value, np.float32)
    nc = build_program()
    in_maps = make_in_maps(query, key, value,
                           np.asarray(Wq, np.float32),
                           np.asarray(Wk, np.float32),
                           np.asarray(Wv, np.float32),
                           np.asarray(bq, np.float32),
                           np.asarray(bk, np.float32))
    res = run_bass_kernel_spmd(nc, in_maps, list(range(8)), trace=_trace)
    out = gather(res.results, bv)
    if _trace:
        return out, res
    return out


# revision 33
# speedup vs baseline: 1.2360x; 1.2360x over previous
"""Multi-head attention (B=4, S=1024, D=1024, H=16) on 8 TRN2 NeuronCores.

Sharding: core c = (batch b = c//2, head-group g = c%2). Each core computes
one batch and 8 of the 16 heads (W columns g*512:(g+1)*512), so per-core
inputs are query[b]/key[b]/value[b] (1024x1024) and W[:, cols] (1024x512).

Per-core kernel:
  - x^T via PE transposes (fp32 has no DMA-transpose path)
  - Q^T/K^T = W^T @ x^T and V = x^T.T @ W as float32r matmuls (full PE
    rate; operands must be produced by an f32r-rounding instruction per
    the BIR verifier), pipelined per 128-dcol slice with the head pairs
    that consume them
  - scores [q,k] once per q-tile; exp + row-sum fused on ACT (accum_out,
    bf16 out); no max-subtraction needed (scores ~ N(0,1), exp safe)
  - attn out = exp * (1/rowsum) on GpSimd, DMA'd in 2-row-block groups
    issued alternately from SP and GpSimd (SWDGE) to parallelize
    descriptor generation
  - exp^T for the context contraction via PE block transposes (emitted one
    q-tile behind the score matmuls so the in-order PE stream never stalls
    on ACT), drained by DVE
  - context as ctx^T = V_h^T(lhsT) @ exp^T(rhs) in bf16, transposed back
    via PE, scaled by 1/rowsum at drain
  - V bias is folded on the host (attn rows sum to 1 => ctx += bv exactly)
"""

import numpy as np
from contextlib import ExitStack

import concourse.bass as bass
import concourse.tile as tile
from concourse import bacc, mybir
from concourse.bass_utils import run_bass_kernel_spmd
from concourse.masks import make_identity

P = 128
S = 1024      # tokens per batch
D = 1024      # model dim
HC = 8        # heads per core
DH = 64       # head dim
WC = 512      # W columns per core
KS = D // P   # 8
MS = WC // P  # 4
TS = S // P   # 8
NT = 512      # matmul moving-dim tile

F32 = mybir.dt.float32
F32R = mybir.dt.float32r
BF16 = mybir.dt.bfloat16
EXP = mybir.ActivationFunctionType.Exp


def _body(ctx, tc, nc, xq_d, xk_d, xv_d, wq_d, wk_d, wv_d, bq_d, bk_d,
          ctx_d, attn_d):
    consts = ctx.enter_context(tc.tile_pool(name="consts", bufs=1))
    wpool = ctx.enter_context(tc.tile_pool(name="w", bufs=2))
    wstage = ctx.enter_context(tc.tile_pool(name="wstage", bufs=1))
    xrow = ctx.enter_context(tc.tile_pool(name="xrow", bufs=2))
    xt = ctx.enter_context(tc.tile_pool(name="xt", bufs=2))
    proj = ctx.enter_context(tc.tile_pool(name="proj", bufs=1))
    hp = ctx.enter_context(tc.tile_pool(name="hp", bufs=2))
    psA = ctx.enter_context(tc.tile_pool(name="psA", bufs=2, space="PSUM"))
    psT = ctx.enter_context(tc.tile_pool(name="psT", bufs=2, space="PSUM"))

    ident = consts.tile([P, P], F32)
    make_identity(nc, ident)
    identb = consts.tile([P, P], BF16)
    nc.vector.tensor_copy(out=identb[:], in_=ident[:])

    bqt = consts.tile([P, MS], F32)
    nc.sync.dma_start(bqt[:], bq_d.rearrange("(o p) -> p o", p=P))
    bkt = consts.tile([P, MS], F32)
    nc.sync.dma_start(bkt[:], bk_d.rearrange("(o p) -> p o", p=P))

    def load_w(w_d):
        # fp32r matmul operands must be explicitly rounded to fp32r by their
        # producer (BIR verifier rule): stage the f32 DMA, round via GpSimd.
        w = wpool.tile([P, KS, WC], F32R, tag="w")
        wre = w_d.rearrange("(ko p) m -> p ko m", p=P)
        for g4 in range(4):
            st = wstage.tile([P, KS // 4, WC], F32, tag="wstage")
            nc.sync.dma_start(st[:], wre[:, g4 * 2:(g4 + 1) * 2, :])
            nc.gpsimd.tensor_copy(out=w[:, g4 * 2:(g4 + 1) * 2, :], in_=st[:])
        return w

    def load_xt(x_d):
        """x [S, D] -> x^T in SBUF as [p, ds, s] with d = ds*128 + p."""
        t = xt.tile([P, KS, S], F32R, tag="xt")
        xre = x_d.rearrange("(t p) d -> p t d", p=P)
        for tg in range(TS // 2):
            row = xrow.tile([P, 2, D], F32, tag="xrow")
            nc.sync.dma_start(row[:], xre[:, tg * 2:(tg + 1) * 2, :])
            for ti in range(2):
                t_ = tg * 2 + ti
                for g4 in range(2):
                    pt = psT.tile([P, 4 * P], F32, tag="pst")
                    for j in range(4):
                        ds = g4 * 4 + j
                        nc.tensor.transpose(
                            pt[:, j * P:(j + 1) * P],
                            row[:, ti, ds * P:(ds + 1) * P], ident[:])
                    nc.vector.tensor_copy(
                        out=t[:, g4 * 4:(g4 + 1) * 4, t_ * P:(t_ + 1) * P],
                        in_=pt[:].rearrange("p (a b) -> p a b", a=4))
        return t

    # --- V = (xv @ Wv) in natural [token, dcol] layout, bf16 ---
    wv = load_w(wv_d)
    xtv = load_xt(xv_d)
    vb = proj.tile([P, TS, WC], BF16, tag="vb")
    for t_ in range(TS):
        ps = psA.tile([P, 2 * NT], F32, tag="mm")
        for ko in range(KS):
            nc.tensor.matmul(ps[:, :NT], xtv[:, ko, t_ * P:(t_ + 1) * P],
                             wv[:, ko, :],
                             start=(ko == 0), stop=(ko == KS - 1))
        nc.vector.tensor_copy(out=vb[:, t_, :], in_=ps[:, :NT])

    wq = load_w(wq_d)
    xtq = load_xt(xq_d)
    wk = load_w(wk_d)
    xtk = load_xt(xk_d)

    def project_m(w, xtx, bias, m, name):
        """One [dcol=128] slice of Q^T/K^T: [128, 1024] f32r, bias folded."""
        out = proj.tile([P, S], F32R, tag=name, name=f"{name}{m}", bufs=2)
        ps = psA.tile([P, 2 * NT], F32, tag="mmproj", bufs=1)
        for nt_ in range(S // NT):
            for ko in range(KS):
                nc.tensor.matmul(
                    ps[:, nt_ * NT:(nt_ + 1) * NT],
                    w[:, ko, m * P:(m + 1) * P],
                    xtx[:, ko, nt_ * NT:(nt_ + 1) * NT],
                    start=(ko == 0), stop=(ko == KS - 1))
        nc.vector.tensor_scalar_add(out[:], ps[:], bias[:, m:m + 1])
        return out

    ctxf = proj.tile([P, TS, WC], F32, tag="ctxf")

    def head(h, qTm, kTm):
        po = (h % 2) * DH
        qh = qTm[po:po + DH, :]   # [64, 1024]
        kh = kTm[po:po + DH, :]

        recip = hp.tile([P, TS], F32, tag="recip")
        exT = hp.tile([P, TS, S], BF16, tag="exT", bufs=1)
        atre = attn_d[h].rearrange("(t p) k -> p t k", p=P)

        # scores [q, k] -> exp(bf16) + rowsum -> (a) normalize -> attn out,
        # (b) PE-transpose into exT [k, q] for the context matmul.
        # Transposes for qt are emitted AFTER qt+1's matmuls so the in-order
        # PE stream never waits on ACT's exp.
        def transposes(qt, exb):
            for g4 in range(2):
                pt = psT.tile([P, 4 * P], BF16, tag="pst")
                for j in range(4):
                    kt = g4 * 4 + j
                    nc.tensor.transpose(pt[:, j * P:(j + 1) * P],
                                        exb[:, kt * P:(kt + 1) * P],
                                        identb[:])
                nc.vector.tensor_copy(
                    out=exT[:, g4 * 4:(g4 + 1) * 4, qt * P:(qt + 1) * P],
                    in_=pt[:].rearrange("p (a b) -> p a b", a=4))

        pending = None
        for qg in range(TS // 2):
            dma_eng = nc.sync if qg % 2 == 0 else nc.gpsimd
            at2 = hp.tile([P, 2, S], F32, tag="at", bufs=3)
            Lp = hp.tile([P, 2], F32, tag="Lp")
            exbs = []
            for q2 in range(2):
                qt = qg * 2 + q2
                ps = psA.tile([P, 2 * NT], F32, tag="mm")
                for nk in range(2):
                    nc.tensor.matmul(ps[:, nk * NT:(nk + 1) * NT],
                                     qh[:, qt * P:(qt + 1) * P],
                                     kh[:, nk * NT:(nk + 1) * NT],
                                     start=True, stop=True)
                exb = hp.tile([P, S], BF16, tag="ex", bufs=4)
                nc.scalar.activation(out=exb[:], in_=ps[:], func=EXP,
                                     scale=0.125, accum_out=Lp[:, q2:q2 + 1])
                exbs.append((qt, exb))
                if pending is not None:
                    transposes(*pending)
                pending = (qt, exb)
            nc.vector.reciprocal(out=recip[:, qg * 2:(qg + 1) * 2], in_=Lp[:])
            for q2, (qt, exb) in enumerate(exbs):
                nc.gpsimd.tensor_scalar_mul(at2[:, q2, :], exb[:],
                                            recip[:, qt:qt + 1])
            dma_eng.dma_start(atre[:, qg * 2:(qg + 1) * 2, :], at2[:])
        if pending is not None:
            transposes(*pending)

        # ctx^T = V_h^T @ expT (bf16), then PE-transpose back + 1/L scale
        for nq in range(2):
            psc_full = psA.tile([P, 2 * NT], F32, tag="mm", name="psc")
            psc = psc_full[:, :NT]
            for kt in range(TS):
                nc.tensor.matmul(psc[:DH], vb[:, kt, h * DH:(h + 1) * DH],
                                 exT[:, kt, nq * NT:(nq + 1) * NT],
                                 start=(kt == 0), stop=(kt == TS - 1))
            ctb = hp.tile([DH, NT], BF16, tag="ctb")
            nc.vector.tensor_copy(out=ctb[:], in_=psc[:DH])
            for j in range(4):
                pt2 = psT.tile([P, P], BF16, tag="pst")
                nc.tensor.transpose(pt2[:, :DH], ctb[:, j * P:(j + 1) * P],
                                    identb[:DH, :DH])
                qt = nq * 4 + j
                nc.vector.tensor_scalar_mul(
                    ctxf[:, qt, h * DH:(h + 1) * DH], pt2[:, :DH],
                    recip[:, qt:qt + 1])

    # m-pipelined: project one dcol-slice of Q^T/K^T, then run its 2 heads,
    # so the scalar engine starts exp work while later slices still project.
    pending_proj = (project_m(wq, xtq, bqt, 0, "qTm"),
                    project_m(wk, xtk, bkt, 0, "kTm"))
    for m in range(MS):
        qTm, kTm = pending_proj
        head(2 * m, qTm, kTm)
        if m + 1 < MS:
            pending_proj = (project_m(wq, xtq, bqt, m + 1, "qTm"),
                            project_m(wk, xtk, bkt, m + 1, "kTm"))
        head(2 * m + 1, qTm, kTm)

    nc.sync.dma_start(ctx_d.rearrange("(t p) d -> p t d", p=P), ctxf[:])


_NC_CACHE = None


def build_program():
    global _NC_CACHE
    if _NC_CACHE is not None:
        return _NC_CACHE
    nc = bacc.Bacc("TRN2", target_bir_lowering=False, debug=False)
    xq = nc.dram_tensor("xq", [S, D], F32, kind="ExternalInput").ap()
    xk = nc.dram_tensor("xk", [S, D], F32, kind="ExternalInput").ap()
    xv = nc.dram_tensor("xv", [S, D], F32, kind="ExternalInput").ap()
    wq = nc.dram_tensor("wq", [D, WC], F32, kind="ExternalInput").ap()
    wk = nc.dram_tensor("wk", [D, WC], F32, kind="ExternalInput").ap()
    wv = nc.dram_tensor("wv", [D, WC], F32, kind="ExternalInput").ap()
    bq = nc.dram_tensor("bq", [WC], F32, kind="ExternalInput").ap()
    bk = nc.dram_tensor("bk", [WC], F32, kind="ExternalInput").ap()
    ctx_d = nc.dram_tensor("ctx", [S, WC], F32, kind="ExternalOutput").ap()
    attn_d = nc.dram_tensor("attn", [HC, S, S], F32,
                            kind="ExternalOutput").ap()
    with tile.TileContext(nc) as tc, ExitStack() as ctx:
        _body(ctx, tc, nc, xq, xk, xv, wq, wk, wv, bq, bk, ctx_d, attn_d)
    nc.compile()
    _NC_CACHE = nc
    return nc


def make_in_maps(query, key, value, Wq, Wk, Wv, bq, bk):
    in_maps = []
    for c in range(8):
        b, g = divmod(c, 2)
        cs = slice(g * WC, (g + 1) * WC)
        in_maps.append({
            "xq": np.ascontiguousarray(query[b]),
            "xk": np.ascontiguousarray(key[b]),
            "xv": np.ascontiguousarray(value[b]),
            "wq": np.ascontiguousarray(Wq[:, cs]),
            "wk": np.ascontiguousarray(Wk[:, cs]),
            "wv": np.ascontiguousarray(Wv[:, cs]),
            "bq": np.ascontiguousarray(bq[cs]),
            "bk": np.ascontiguousarray(bk[cs]),
        })
    return in_maps


def gather(results, bv):
    B, H = 4, 16
    context = np.empty((B, S, D), np.float32)
    attn = np.empty((H * B, S, S), np.float32)
    for c in range(8):
        b, g = divmod(c, 2)
        context[b, :, g * WC:(g + 1) * WC] = results[c]["ctx"]
        for hl in range(HC):
            attn[(g * HC + hl) * B + b] = results[c]["attn"][hl]
    context += np.asarray(bv, np.float32)  # exact: attn rows sum to 1
    return context, attn


def kernel(query, key, value, Wq, bq, Wk, bk, Wv, bv, _trace=False):
    query = np.asarray(query, np.float32)
    key = np.asarray(key, np.float32)
    value = np.asarray(value, np.float32)
    nc = build_program()
    in_maps = make_in_maps(query, key, value,
                           np.asarray(Wq, np.float32),
                           np.asarray(Wk, np.float32),
                           np.asarray(Wv, np.float32),
                           np.asarray(bq, np.float32),
                           np.asarray(bk, np.float32))
    res = run_bass_kernel_spmd(nc, in_maps, list(range(8)), trace=_trace)
    out = gather(res.results, bv)
    if _trace:
        return out, res
    return out


# revision 44
# speedup vs baseline: 1.8651x; 1.5090x over previous
"""Multi-head attention (B=4, S=1024, D=1024, H=16) on 8 TRN2 NeuronCores.

Sharding: core c = (batch b = c//2, head-group g = c%2). Each core computes
one batch and 8 of the 16 heads (W columns g*512:(g+1)*512), so per-core
inputs are query[b]/key[b]/value[b] (1024x1024) and W[:, cols] (1024x512).

Per-core kernel:
  - x^T via PE transposes (fp32 has no DMA-transpose path)
  - Q^T/K^T = W^T @ x^T and V = x^T.T @ W as float32r matmuls (full PE
    rate; operands must be produced by an f32r-rounding instruction per
    the BIR verifier), pipelined per 128-dcol slice with the head pairs
    that consume them
  - scores [q,k] once per q-tile; exp + row-sum fused on ACT (accum_out,
    bf16 out); no max-subtraction needed (scores ~ N(0,1), exp safe)
  - attn out = exp * (1/rowsum) on GpSimd, DMA'd in 2-row-block groups
    issued alternately from SP and GpSimd (SWDGE) to parallelize
    descriptor generation
  - exp^T for the context contraction via PE block transposes (emitted one
    q-tile behind the score matmuls so the in-order PE stream never stalls
    on ACT), drained by DVE
  - context as ctx^T = V_h^T(lhsT) @ exp^T(rhs) in bf16, transposed back
    via PE, scaled by 1/rowsum at drain
  - V bias is folded on the host (attn rows sum to 1 => ctx += bv exactly)
"""

import numpy as np
from contextlib import ExitStack

import concourse.bass as bass
import concourse.tile as tile
from concourse import bacc, mybir
from concourse.bass_utils import run_bass_kernel_spmd
from concourse.masks import make_identity

P = 128
S = 1024      # tokens per batch
D = 1024      # model dim
HC = 8        # heads per core
DH = 64       # head dim
WC = 512      # W columns per core
KS = D // P   # 8
MS = WC // P  # 4
TS = S // P   # 8
NT = 512      # matmul moving-dim tile

F32 = mybir.dt.float32
F32R = mybir.dt.float32r
BF16 = mybir.dt.bfloat16
EXP = mybir.ActivationFunctionType.Exp


def _body(ctx, tc, nc, xq_d, xk_d, xv_d, wq_d, wk_d, wv_d, bq_d, bk_d,
          ctx_d, attn_d):
    consts = ctx.enter_context(tc.tile_pool(name="consts", bufs=1))
    wpool = ctx.enter_context(tc.tile_pool(name="w", bufs=2))
    wstage = ctx.enter_context(tc.tile_pool(name="wstage", bufs=1))
    xrow = ctx.enter_context(tc.tile_pool(name="xrow", bufs=2))
    xt = ctx.enter_context(tc.tile_pool(name="xt", bufs=2))
    proj = ctx.enter_context(tc.tile_pool(name="proj", bufs=1))
    hp = ctx.enter_context(tc.tile_pool(name="hp", bufs=2))
    psA = ctx.enter_context(tc.tile_pool(name="psA", bufs=2, space="PSUM"))
    psT = ctx.enter_context(tc.tile_pool(name="psT", bufs=2, space="PSUM"))

    ident = consts.tile([P, P], F32)
    make_identity(nc, ident)
    identb = consts.tile([P, P], BF16)
    nc.vector.tensor_copy(out=identb[:], in_=ident[:])

    bqt = consts.tile([P, MS], F32)
    nc.sync.dma_start(bqt[:], bq_d.rearrange("(o p) -> p o", p=P))
    bkt = consts.tile([P, MS], F32)
    nc.sync.dma_start(bkt[:], bk_d.rearrange("(o p) -> p o", p=P))

    def load_w(w_d):
        # fp32r matmul operands must be explicitly rounded to fp32r by their
        # producer (BIR verifier rule): stage the f32 DMA, round via GpSimd.
        w = wpool.tile([P, KS, WC], F32R, tag="w")
        wre = w_d.rearrange("(ko p) m -> p ko m", p=P)
        for g4 in range(4):
            st = wstage.tile([P, KS // 4, WC], F32, tag="wstage")
            nc.sync.dma_start(st[:], wre[:, g4 * 2:(g4 + 1) * 2, :])
            nc.gpsimd.tensor_copy(out=w[:, g4 * 2:(g4 + 1) * 2, :], in_=st[:])
        return w

    def load_xt(x_d):
        """x [S, D] -> x^T in SBUF as [p, ds, s] with d = ds*128 + p."""
        t = xt.tile([P, KS, S], F32R, tag="xt")
        xre = x_d.rearrange("(t p) d -> p t d", p=P)
        for tg in range(TS // 2):
            row = xrow.tile([P, 2, D], F32, tag="xrow")
            nc.sync.dma_start(row[:], xre[:, tg * 2:(tg + 1) * 2, :])
            for ti in range(2):
                t_ = tg * 2 + ti
                for g4 in range(2):
                    pt = psT.tile([P, 4 * P], F32, tag="pst")
                    for j in range(4):
                        ds = g4 * 4 + j
                        nc.tensor.transpose(
                            pt[:, j * P:(j + 1) * P],
                            row[:, ti, ds * P:(ds + 1) * P], ident[:])
                    nc.vector.tensor_copy(
                        out=t[:, g4 * 4:(g4 + 1) * 4, t_ * P:(t_ + 1) * P],
                        in_=pt[:].rearrange("p (a b) -> p a b", a=4))
        return t

    # --- V = (xv @ Wv) in natural [token, dcol] layout, bf16 ---
    wv = load_w(wv_d)
    xtv = load_xt(xv_d)
    vb = proj.tile([P, TS, WC], BF16, tag="vb")
    for t_ in range(TS):
        ps = psA.tile([P, 2 * NT], F32, tag="mm")
        for ko in range(KS):
            nc.tensor.matmul(ps[:, :NT], xtv[:, ko, t_ * P:(t_ + 1) * P],
                             wv[:, ko, :],
                             start=(ko == 0), stop=(ko == KS - 1))
        nc.vector.tensor_copy(out=vb[:, t_, :], in_=ps[:, :NT])

    wq = load_w(wq_d)
    xtq = load_xt(xq_d)
    wk = load_w(wk_d)
    xtk = load_xt(xk_d)

    def project_m(w, xtx, bias, m, name):
        """One [dcol=128] slice of Q^T/K^T: [128, 1024] f32r, bias folded."""
        out = proj.tile([P, S], F32R, tag=name, name=f"{name}{m}", bufs=2)
        ps = psA.tile([P, 2 * NT], F32, tag="mmproj", bufs=1)
        for nt_ in range(S // NT):
            for ko in range(KS):
                nc.tensor.matmul(
                    ps[:, nt_ * NT:(nt_ + 1) * NT],
                    w[:, ko, m * P:(m + 1) * P],
                    xtx[:, ko, nt_ * NT:(nt_ + 1) * NT],
                    start=(ko == 0), stop=(ko == KS - 1))
        nc.vector.tensor_scalar_add(out[:], ps[:], bias[:, m:m + 1])
        return out

    ctxf = proj.tile([P, TS, WC], F32, tag="ctxf")

    def head(h, qTm, kTm):
        po = (h % 2) * DH
        qh = qTm[po:po + DH, :]   # [64, 1024]
        kh = kTm[po:po + DH, :]

        recip = hp.tile([P, TS], F32, tag="recip")
        exT = hp.tile([P, TS, S], BF16, tag="exT", bufs=1)
        atre = attn_d[h].rearrange("(t p) k -> p t k", p=P)

        # scores [q, k] -> exp(bf16) + rowsum -> (a) normalize -> attn out,
        # (b) PE-transpose into exT [k, q] for the context matmul.
        # Transposes for qt are emitted AFTER qt+1's matmuls so the in-order
        # PE stream never waits on ACT's exp.
        def transposes(qt, exb):
            for g4 in range(2):
                pt = psT.tile([P, 4 * P], BF16, tag="pst")
                for j in range(4):
                    kt = g4 * 4 + j
                    nc.tensor.transpose(pt[:, j * P:(j + 1) * P],
                                        exb[:, kt * P:(kt + 1) * P],
                                        identb[:])
                nc.vector.tensor_copy(
                    out=exT[:, g4 * 4:(g4 + 1) * 4, qt * P:(qt + 1) * P],
                    in_=pt[:].rearrange("p (a b) -> p a b", a=4))

        pending = None
        for qg in range(TS // 2):
            dma_eng = nc.sync
            at2 = hp.tile([P, 2, S], F32, tag="at", bufs=3)
            Lp = hp.tile([P, 2], F32, tag="Lp")
            exbs = []
            for q2 in range(2):
                qt = qg * 2 + q2
                ps = psA.tile([P, 2 * NT], F32, tag="mm")
                for nk in range(2):
                    nc.tensor.matmul(ps[:, nk * NT:(nk + 1) * NT],
                                     qh[:, qt * P:(qt + 1) * P],
                                     kh[:, nk * NT:(nk + 1) * NT],
                                     start=True, stop=True)
                exb = hp.tile([P, S], BF16, tag="ex", bufs=4)
                nc.scalar.activation(out=exb[:], in_=ps[:], func=EXP,
                                     scale=0.125, accum_out=Lp[:, q2:q2 + 1])
                exbs.append((qt, exb))
                if pending is not None:
                    transposes(*pending)
                pending = (qt, exb)
            nc.vector.reciprocal(out=recip[:, qg * 2:(qg + 1) * 2], in_=Lp[:])
            for q2, (qt, exb) in enumerate(exbs):
                neng = nc.gpsimd if (qg * 2 + q2) % 4 else nc.vector
                neng.tensor_scalar_mul(at2[:, q2, :], exb[:],
                                       recip[:, qt:qt + 1])
            dma_eng.dma_start(atre[:, qg * 2:(qg + 1) * 2, :], at2[:])
        if pending is not None:
            transposes(*pending)

        # ctx^T = V_h^T @ expT (bf16), then PE-transpose back + 1/L scale
        for nq in range(2):
            psc_full = psA.tile([P, 2 * NT], F32, tag="mm", name="psc")
            psc = psc_full[:, :NT]
            for kt in range(TS):
                nc.tensor.matmul(psc[:DH], vb[:, kt, h * DH:(h + 1) * DH],
                                 exT[:, kt, nq * NT:(nq + 1) * NT],
                                 start=(kt == 0), stop=(kt == TS - 1))
            ctb = hp.tile([DH, NT], BF16, tag="ctb")
            nc.vector.tensor_copy(out=ctb[:], in_=psc[:DH])
            for j in range(4):
                pt2 = psT.tile([P, P], BF16, tag="pst")
                nc.tensor.transpose(pt2[:, :DH], ctb[:, j * P:(j + 1) * P],
                                    identb[:DH, :DH])
                qt = nq * 4 + j
                nc.vector.tensor_scalar_mul(
                    ctxf[:, qt, h * DH:(h + 1) * DH], pt2[:, :DH],
                    recip[:, qt:qt + 1])

    # m-pipelined: project one dcol-slice of Q^T/K^T, then run its 2 heads,
    # so the scalar engine starts exp work while later slices still project.
    pending_proj = (project_m(wq, xtq, bqt, 0, "qTm"),
                    project_m(wk, xtk, bkt, 0, "kTm"))
    for m in range(MS):
        qTm, kTm = pending_proj
        head(2 * m, qTm, kTm)
        if m + 1 < MS:
            pending_proj = (project_m(wq, xtq, bqt, m + 1, "qTm"),
                            project_m(wk, xtk, bkt, m + 1, "kTm"))
        head(2 * m + 1, qTm, kTm)

    nc.sync.dma_start(ctx_d.rearrange("(t p) d -> p t d", p=P), ctxf[:])


_NC_CACHE = None


def build_program():
    global _NC_CACHE
    if _NC_CACHE is not None:
        return _NC_CACHE
    nc = bacc.Bacc("TRN2", target_bir_lowering=False, debug=False)
    xq = nc.dram_tensor("xq", [S, D], F32, kind="ExternalInput").ap()
    xk = nc.dram_tensor("xk", [S, D], F32, kind="ExternalInput").ap()
    xv = nc.dram_tensor("xv", [S, D], F32, kind="ExternalInput").ap()
    wq = nc.dram_tensor("wq", [D, WC], F32, kind="ExternalInput").ap()
    wk = nc.dram_tensor("wk", [D, WC], F32, kind="ExternalInput").ap()
    wv = nc.dram_tensor("wv", [D, WC], F32, kind="ExternalInput").ap()
    bq = nc.dram_tensor("bq", [WC], F32, kind="ExternalInput").ap()
    bk = nc.dram_tensor("bk", [WC], F32, kind="ExternalInput").ap()
    ctx_d = nc.dram_tensor("ctx", [S, WC], F32, kind="ExternalOutput").ap()
    attn_d = nc.dram_tensor("attn", [HC, S, S], F32,
                            kind="ExternalOutput").ap()
    with tile.TileContext(nc) as tc, ExitStack() as ctx:
        _body(ctx, tc, nc, xq, xk, xv, wq, wk, wv, bq, bk, ctx_d, attn_d)
    nc.compile()
    _NC_CACHE = nc
    return nc


def make_in_maps(query, key, value, Wq, Wk, Wv, bq, bk):
    in_maps = []
    for c in range(8):
        b, g = divmod(c, 2)
        cs = slice(g * WC, (g + 1) * WC)
        in_maps.append({
            "xq": np.ascontiguousarray(query[b]),
            "xk": np.ascontiguousarray(key[b]),
            "xv": np.ascontiguousarray(value[b]),
            "wq": np.ascontiguousarray(Wq[:, cs]),
            "wk": np.ascontiguousarray(Wk[:, cs]),
            "wv": np.ascontiguousarray(Wv[:, cs]),
            "bq": np.ascontiguousarray(bq[cs]),
            "bk": np.ascontiguousarray(bk[cs]),
        })
    return in_maps


def gather(results, bv):
    B, H = 4, 16
    context = np.empty((B, S, D), np.float32)
    attn = np.empty((H * B, S, S), np.float32)
    for c in range(8):
        b, g = divmod(c, 2)
        context[b, :, g * WC:(g + 1) * WC] = results[c]["ctx"]
        for hl in range(HC):
            attn[(g * HC + hl) * B + b] = results[c]["attn"][hl]
    context += np.asarray(bv, np.float32)  # exact: attn rows sum to 1
    return context, attn


def kernel(query, key, value, Wq, bq, Wk, bk, Wv, bv, _trace=False):
    query = np.asarray(query, np.float32)
    key = np.asarray(key, np.float32)
    value = np.asarray(value, np.float32)
    nc = build_program()
    in_maps = make_in_maps(query, key, value,
                           np.asarray(Wq, np.float32),
                           np.asarray(Wk, np.float32),
                           np.asarray(Wv, np.float32),
                           np.asarray(bq, np.float32),
                           np.asarray(bk, np.float32))
    res = run_bass_kernel_spmd(nc, in_maps, list(range(8)), trace=_trace)
    out = gather(res.results, bv)
    if _trace:
        return out, res
    return out
